# revision 19
# baseline (speedup 1.0000x reference)
"""CrossCompress unit kernel for Trainium2, 8-core data parallel.

Reference computation (per batch row b, D=128):
    item_out[b]   = v[b] * (e[b]@w_vv) + e[b] * (v[b]@w_ev) + bias_v
    entity_out[b] = v[b] * (e[b]@w_ve) + e[b] * (v[b]@w_ee) + bias_e

Strategy: pure data parallel over B=16384 rows -> 2048 rows/core.
Each core works in a transposed layout [D=128 partitions, batch free]:
the four per-row dot products become PE matmuls whose stationary operand
is the (D,1) weight replicated across 128 columns -- one matmul both
computes the dots AND broadcasts the result down all partitions.

All I/O and SBUF data is fp16 (PE fp16 1 cycle/row; DMA bytes halve).
PSUM accumulation stays fp32. Global rel error ~1e-3 (gate 2e-2).

Supertiles are sized [128, 896, 896, 128]: a small first tile so the
DVE starts ~2us earlier (DMA latency chain is ~2.3us), big middle tiles
for throughput (the DVE TT-mul's 250ns PSUM-access overhead amortizes),
and a small last tile so the pipeline drain is short. Per supertile:
  PE   : 4 dot+broadcast matmuls into two 2-bank psum pair tiles
  DVE  : 2 pair-packed products t_v = v (*) sA, t_e = e (*) sB (the only
         PSUM-capable tensor-tensor engine -> structural ~9.5us floor)
  Pool : pair-packed add ts = t_v + t_e  (st0..st2)
  Act  : per-half bias add into the output tile (st0..st2)
  last : (t_v+bias)+t_e via scalar_tensor_tensor, one half on DVE and
         one on GpSimd concurrently -> ~0.2us drain instead of ~2us
Output halves ride ONE merged DMA per supertile, issued from the GpSimd
queue (SWDGE, ~25ns engine cost) so the Sync engine's 565ns/DMA issue
serialization stays off the tail.

Walrus CoreV3 codegen accepts only ONE embedded sync wait per
instruction; a post-pass splits any multi-wait instruction into
single-wait NoOps.
"""
import sys
sys.path.insert(0, '/opt/trn_rl_repo')
import numpy as np
import bass_rust
import concourse.bass as bass
import concourse.tile as tile
from concourse import mybir
from concourse.bass_utils import run_bass_kernel_spmd

B, D = 16384, 128
NCORES = 8
RPC = B // NCORES          # rows per core = 2048
NS = (256, 512, 512, 384, 384)  # supertile batch-column counts
WARMUP_MM = 0              # PE p-state warmup matmuls (0: inputs are prefetched)
assert sum(NS) == RPC
CW = 4 * D + 2             # const block: 4 replicated weights + 2 biases

F32 = mybir.dt.float32
F16 = mybir.dt.float16


def _build():
    nc = bass.Bass("TRN2", target_bir_lowering=False, debug=False,
                   num_devices=NCORES)
    # flat input per core: [D, CW + 2*RPC]: [consts | st0 v|e | st1 v|e ...]
    xin = nc.dram_tensor("xin", [D, CW + 2 * RPC], F16,
                         kind="ExternalInput").ap()
    out = nc.dram_tensor("out", [D, 2 * RPC], F16, kind="ExternalOutput").ap()

    nst = len(NS)
    with tile.TileContext(nc) as tc:
        with tc.tile_pool(name="c0", bufs=1) as c0_pool, \
             tc.tile_pool(name="io", bufs=6) as io_pool, \
             tc.tile_pool(name="ob", bufs=3) as ob_pool, \
             tc.tile_pool(name="tmp", bufs=3) as tmp_pool, \
             tc.tile_pool(name="ps", bufs=2, space="PSUM") as ps_pool:

            # consts alone in a tiny first DMA so matmuls unblock early
            c0_sb = c0_pool.tile([D, CW], F16)
            nc.sync.dma_start(out=c0_sb[:], in_=xin[:, 0:CW])
            # PE p-state warmup: matmuls on a never-DMA'd scratch tile (no
            # producer -> no waits) keep the PE continuously busy through
            # the preamble+DMA window so real matmuls run at full clock.
            if WARMUP_MM:
                wup = c0_pool.tile([D, 2 * D], F16, tag="wup")
                nc.gpsimd.memset(wup[:], 0.0)
            w_sb = c0_sb[:, 0:4 * D]
            bv_sb = c0_sb[:, 4 * D:4 * D + 1]
            be_sb = c0_sb[:, 4 * D + 1:CW]

            if WARMUP_MM:
                wps = ps_pool.tile([D, 2, 2, 512], F32, tag="sAB",
                                   name="warmup_ps")
                for k in range(WARMUP_MM):
                    nc.tensor.matmul(wps[:, k % 2, k // 2 % 2, 0:2 * D],
                                     wup[:, 0:D], wup[:], start=True,
                                     stop=True)

            in_off = CW
            out_off = 0
            for st, N in enumerate(NS):
                ve_sb = io_pool.tile([D, 2 * N], F16, tag="ve",
                                     name=f"ve_{st}")
                nc.sync.dma_start(out=ve_sb[:],
                                  in_=xin[:, in_off:in_off + 2 * N])
                in_off += 2 * N
                v_sb = ve_sb[:, 0:N]
                e_sb = ve_sb[:, N:2 * N]

                # dot+broadcast matmuls, one 4-slot psum tile:
                # sAB = [e@w_vv | e@w_ve | v@w_ev | v@w_ee] = [a|b|c|d]
                # each dot-product slot gets a FULL 2KB psum bank (a
                # matmul's psum output must not cross a bank boundary);
                # only the first N columns of each bank are written/read
                sAB = ps_pool.tile([D, 2, 2, 512], F32, tag="sAB",
                                   name=f"sAB_{st}")
                nc.tensor.matmul(sAB[:, 0, 0, 0:N], w_sb[:, 0 * D:1 * D],
                                 e_sb, start=True, stop=True)
                nc.tensor.matmul(sAB[:, 0, 1, 0:N], w_sb[:, 2 * D:3 * D],
                                 e_sb, start=True, stop=True)
                nc.tensor.matmul(sAB[:, 1, 0, 0:N], w_sb[:, 1 * D:2 * D],
                                 v_sb, start=True, stop=True)
                nc.tensor.matmul(sAB[:, 1, 1, 0:N], w_sb[:, 3 * D:4 * D],
                                 v_sb, start=True, stop=True)

                # quad-packed product on DVE: t = [v,v,e,e] (*) sAB
                # (in0 reads ve_sb as [D, {v,e}, x2, N] with a stride-0 dim);
                # the FIRST tile splits into two pair-products so the DVE
                # starts right after the first two matmuls land
                t = tmp_pool.tile([D, 2, 2, N], F16, tag="t", name=f"t_{st}")
                in0 = ve_sb.rearrange("p (b n) -> p b n", b=2).unsqueeze(
                    2).broadcast_to([D, 2, 2, N])
                if st == 0:
                    nc.vector.tensor_mul(t[:, 0], in0[:, 0],
                                         sAB[:, 0, :, 0:N])
                    nc.vector.tensor_mul(t[:, 1], in0[:, 1],
                                         sAB[:, 1, :, 0:N])
                else:
                    nc.vector.tensor_mul(t[:], in0, sAB[:, :, :, 0:N])
                t_v = t[:, 0]     # [a*v | b*v]
                t_e = t[:, 1]     # [c*e | d*e]

                o_sb = ob_pool.tile([D, 2, N], F16, tag="o", name=f"o_{st}")
                if st < nst - 1:
                    # half-granular add->bias->DMA so the tail of each tile
                    # streams out while the other half is still in flight
                    ts = tmp_pool.tile([D, 2, N], F16, tag="ts",
                                       name=f"ts_{st}")
                    for h, bias in ((0, bv_sb), (1, be_sb)):
                        nc.gpsimd.tensor_add(ts[:, h], t[:, 0, h], t[:, 1, h])
                        nc.scalar.activation(
                            o_sb[:, h], ts[:, h],
                            mybir.ActivationFunctionType.Identity,
                            bias=bias, scale=1.0)
                        nc.sync.dma_start(
                            out=out[:, out_off + h * N:out_off + (h + 1) * N],
                            in_=o_sb[:, h])
                else:
                    # last tile: fused (t_v+bias)+t_e on DVE, half DMAs
                    # issued from the idle GpSimd queue as each half lands
                    nc.vector.scalar_tensor_tensor(
                        o_sb[:, 0], t[:, 0, 0], bv_sb, t[:, 1, 0],
                        op0=mybir.AluOpType.add, op1=mybir.AluOpType.add)
                    nc.gpsimd.dma_start(out=out[:, out_off:out_off + N],
                                        in_=o_sb[:, 0])
                    nc.vector.scalar_tensor_tensor(
                        o_sb[:, 1], t[:, 0, 1], be_sb, t[:, 1, 1],
                        op0=mybir.AluOpType.add, op1=mybir.AluOpType.add)
                    nc.scalar.dma_start(
                        out=out[:, out_off + N:out_off + 2 * N],
                        in_=o_sb[:, 1])
                out_off += 2 * N
    _split_multiwaits(nc)
    _hoist_first_dmas(nc, k=1 + len(NS))
    return nc


def _hoist_first_dmas(nc, k):
    """Move the first k wait-free SP DMA instructions to the very front of
    the first block, ahead of the framework preamble barriers, so their
    ~2.3us issue+transfer+semaphore latency hides under the ~7us engine
    spin-up window. Their semaphore updates are unchanged -- consumers
    still wait on the same counts."""
    blocks = nc.m.functions[0].blocks
    moved = []
    for b in blocks:
        insts = b.instructions
        keep = []
        for inst in insts:
            if (len(moved) < k and 'DMA' in str(inst.opcode)
                    and inst.engine == mybir.EngineType.SP
                    and (inst.sync_info is None
                         or not inst.sync_info.on_wait)):
                moved.append(inst)
            else:
                keep.append(inst)
        insts[:] = keep
        if len(moved) >= k:
            break
    blocks[0].instructions[:0] = moved
    return len(moved)


def _split_multiwaits(nc):
    """Split instructions carrying >1 sync wait into single-wait NoOps
    inserted just before them on the same engine queue."""
    n = 0
    for b in nc.m.functions[0].blocks:
        insts = b.instructions
        new = []
        for inst in insts:
            si = inst.sync_info
            if si is not None and si.on_wait and len(si.on_wait) > 1:
                waits = list(si.on_wait)
                for k, w in enumerate(waits[:-1]):
                    nop = mybir.InstNoOp(name=f"{inst.name}-sw{k}",
                                         ins=[], outs=[])
                    nop.engine = inst.engine
                    nop.sync_info = bass_rust.SyncInfo(on_wait=[w],
                                                       on_update=[])
                    nc.register_instruction(nop)
                    new.append(nop)
                    n += 1
                si.on_wait = [waits[-1]]
            new.append(inst)
        insts[:] = new
    return n


_NC = None


def _get_nc():
    global _NC
    if _NC is None:
        _NC = _build()
    return _NC


def _make_in_maps(v, e, w_vv, w_ve, w_ev, w_ee, bias_v, bias_e):
    cst = np.empty((D, CW), np.float16)
    cst[:, 0 * D:1 * D] = np.repeat(w_vv.reshape(D, 1), D, axis=1)
    cst[:, 1 * D:2 * D] = np.repeat(w_ev.reshape(D, 1), D, axis=1)
    cst[:, 2 * D:3 * D] = np.repeat(w_ve.reshape(D, 1), D, axis=1)
    cst[:, 3 * D:4 * D] = np.repeat(w_ee.reshape(D, 1), D, axis=1)
    cst[:, 4 * D] = bias_v.reshape(D)
    cst[:, 4 * D + 1] = bias_e.reshape(D)

    vT = np.ascontiguousarray(v.T).astype(np.float16)   # [D, B]
    eT = np.ascontiguousarray(e.T).astype(np.float16)
    in_maps = []
    for c in range(NCORES):
        xin = np.empty((D, CW + 2 * RPC), np.float16)
        xin[:, 0:CW] = cst
        base = c * RPC
        off = CW
        lo = base
        for N in NS:
            xin[:, off:off + N] = vT[:, lo:lo + N]
            xin[:, off + N:off + 2 * N] = eT[:, lo:lo + N]
            off += 2 * N
            lo += N
        in_maps.append({"xin": xin})
    return in_maps


def _run(in_maps, trace=False):
    return run_bass_kernel_spmd(_get_nc(), in_maps, list(range(NCORES)),
                                trace=trace)


def kernel(item_embedding, entity_embedding, w_vv, w_ve, w_ev, w_ee,
           bias_v, bias_e, _trace=False, _res_out=None):
    v = np.asarray(item_embedding, np.float32).reshape(B, D)
    e = np.asarray(entity_embedding, np.float32).reshape(B, D)
    in_maps = _make_in_maps(
        v, e,
        np.asarray(w_vv, np.float32), np.asarray(w_ve, np.float32),
        np.asarray(w_ev, np.float32), np.asarray(w_ee, np.float32),
        np.asarray(bias_v, np.float32), np.asarray(bias_e, np.float32))
    res = _run(in_maps, trace=_trace)
    if _res_out is not None:
        _res_out.append(res)
    item = np.empty((B, D, 1), np.float32)
    ent = np.empty((B, D, 1), np.float32)
    for c in range(NCORES):
        o = res.results[c]["out"]            # [D, 2*RPC] fp16
        base = c * RPC
        off = 0
        lo = base
        for N in NS:
            item[lo:lo + N, :, 0] = o[:, off:off + N].T
            ent[lo:lo + N, :, 0] = o[:, off + N:off + 2 * N].T
            off += 2 * N
            lo += N
    return (item, ent)


# revision 20
# speedup vs baseline: 1.0158x; 1.0158x over previous
"""CrossCompress unit kernel for Trainium2, 8-core data parallel.

Reference computation (per batch row b, D=128):
    item_out[b]   = v[b] * (e[b]@w_vv) + e[b] * (v[b]@w_ev) + bias_v
    entity_out[b] = v[b] * (e[b]@w_ve) + e[b] * (v[b]@w_ee) + bias_e

Strategy: pure data parallel over B=16384 rows -> 2048 rows/core.
Each core works in a transposed layout [D=128 partitions, batch free]:
the four per-row dot products become PE matmuls whose stationary operand
is the (D,1) weight replicated across 128 columns -- one matmul both
computes the dots AND broadcasts the result down all partitions.

All I/O and SBUF data is fp16 (PE fp16 1 cycle/row; DMA bytes halve).
PSUM accumulation stays fp32. Global rel error ~1e-3 (gate 2e-2).

Supertiles are sized [128, 896, 896, 128]: a small first tile so the
DVE starts ~2us earlier (DMA latency chain is ~2.3us), big middle tiles
for throughput (the DVE TT-mul's 250ns PSUM-access overhead amortizes),
and a small last tile so the pipeline drain is short. Per supertile:
  PE   : 4 dot+broadcast matmuls into two 2-bank psum pair tiles
  DVE  : 2 pair-packed products t_v = v (*) sA, t_e = e (*) sB (the only
         PSUM-capable tensor-tensor engine -> structural ~9.5us floor)
  Pool : pair-packed add ts = t_v + t_e  (st0..st2)
  Act  : per-half bias add into the output tile (st0..st2)
  last : (t_v+bias)+t_e via scalar_tensor_tensor, one half on DVE and
         one on GpSimd concurrently -> ~0.2us drain instead of ~2us
Output halves ride ONE merged DMA per supertile, issued from the GpSimd
queue (SWDGE, ~25ns engine cost) so the Sync engine's 565ns/DMA issue
serialization stays off the tail.

Walrus CoreV3 codegen accepts only ONE embedded sync wait per
instruction; a post-pass splits any multi-wait instruction into
single-wait NoOps.
"""
import sys
sys.path.insert(0, '/opt/trn_rl_repo')
import numpy as np
import bass_rust
import concourse.bass as bass
import concourse.tile as tile
from concourse import mybir
from concourse.bass_utils import run_bass_kernel_spmd

B, D = 16384, 128
NCORES = 8
RPC = B // NCORES          # rows per core = 2048
NS = (256, 512, 512, 384, 384)  # supertile batch-column counts
WARMUP_MM = 0              # PE p-state warmup matmuls (0: inputs are prefetched)
assert sum(NS) == RPC
CW = 4 * D + 2             # const block: 4 replicated weights + 2 biases

F32 = mybir.dt.float32
F16 = mybir.dt.float16


def _build():
    nc = bass.Bass("TRN2", target_bir_lowering=False, debug=False,
                   num_devices=NCORES)
    # flat input per core: [D, CW + 2*RPC]: [consts | st0 v|e | st1 v|e ...]
    xin = nc.dram_tensor("xin", [D, CW + 2 * RPC], F16,
                         kind="ExternalInput").ap()
    out = nc.dram_tensor("out", [D, 2 * RPC], F16, kind="ExternalOutput").ap()

    nst = len(NS)
    with tile.TileContext(nc) as tc:
        with tc.tile_pool(name="c0", bufs=1) as c0_pool, \
             tc.tile_pool(name="io", bufs=6) as io_pool, \
             tc.tile_pool(name="ob", bufs=3) as ob_pool, \
             tc.tile_pool(name="tmp", bufs=3) as tmp_pool, \
             tc.tile_pool(name="ps", bufs=2, space="PSUM") as ps_pool:

            # consts alone in a tiny first DMA so matmuls unblock early
            c0_sb = c0_pool.tile([D, CW], F16)
            nc.sync.dma_start(out=c0_sb[:], in_=xin[:, 0:CW])
            # PE p-state warmup: matmuls on a never-DMA'd scratch tile (no
            # producer -> no waits) keep the PE continuously busy through
            # the preamble+DMA window so real matmuls run at full clock.
            if WARMUP_MM:
                wup = c0_pool.tile([D, 2 * D], F16, tag="wup")
                nc.gpsimd.memset(wup[:], 0.0)
            w_sb = c0_sb[:, 0:4 * D]
            bv_sb = c0_sb[:, 4 * D:4 * D + 1]
            be_sb = c0_sb[:, 4 * D + 1:CW]

            if WARMUP_MM:
                wps = ps_pool.tile([D, 2, 2, 512], F32, tag="sAB",
                                   name="warmup_ps")
                for k in range(WARMUP_MM):
                    nc.tensor.matmul(wps[:, k % 2, k // 2 % 2, 0:2 * D],
                                     wup[:, 0:D], wup[:], start=True,
                                     stop=True)

            in_off = CW
            out_off = 0
            for st, N in enumerate(NS):
                ve_sb = io_pool.tile([D, 2 * N], F16, tag="ve",
                                     name=f"ve_{st}")
                nc.sync.dma_start(out=ve_sb[:],
                                  in_=xin[:, in_off:in_off + 2 * N])
                in_off += 2 * N
                v_sb = ve_sb[:, 0:N]
                e_sb = ve_sb[:, N:2 * N]

                # dot+broadcast matmuls, one 4-slot psum tile:
                # sAB = [e@w_vv | e@w_ve | v@w_ev | v@w_ee] = [a|b|c|d]
                # each dot-product slot gets a FULL 2KB psum bank (a
                # matmul's psum output must not cross a bank boundary);
                # only the first N columns of each bank are written/read
                sAB = ps_pool.tile([D, 2, 2, 512], F32, tag="sAB",
                                   name=f"sAB_{st}")
                nc.tensor.matmul(sAB[:, 0, 0, 0:N], w_sb[:, 0 * D:1 * D],
                                 e_sb, start=True, stop=True)
                nc.tensor.matmul(sAB[:, 0, 1, 0:N], w_sb[:, 2 * D:3 * D],
                                 e_sb, start=True, stop=True)
                nc.tensor.matmul(sAB[:, 1, 0, 0:N], w_sb[:, 1 * D:2 * D],
                                 v_sb, start=True, stop=True)
                nc.tensor.matmul(sAB[:, 1, 1, 0:N], w_sb[:, 3 * D:4 * D],
                                 v_sb, start=True, stop=True)

                # ONE quad-packed product on DVE: t = [v,v,e,e] (*) sAB
                # (in0 reads ve_sb as [D, {v,e}, x2, N] with a stride-0 dim)
                t = tmp_pool.tile([D, 2, 2, N], F16, tag="t", name=f"t_{st}")
                in0 = ve_sb.rearrange("p (b n) -> p b n", b=2).unsqueeze(
                    2).broadcast_to([D, 2, 2, N])
                nc.vector.tensor_mul(t[:], in0, sAB[:, :, :, 0:N])
                t_v = t[:, 0]     # [a*v | b*v]
                t_e = t[:, 1]     # [c*e | d*e]

                o_sb = ob_pool.tile([D, 2, N], F16, tag="o", name=f"o_{st}")
                if st < nst - 1:
                    # half-granular add->bias->DMA so the tail of each tile
                    # streams out while the other half is still in flight
                    ts = tmp_pool.tile([D, 2, N], F16, tag="ts",
                                       name=f"ts_{st}")
                    for h, bias in ((0, bv_sb), (1, be_sb)):
                        nc.gpsimd.tensor_add(ts[:, h], t[:, 0, h], t[:, 1, h])
                        nc.scalar.activation(
                            o_sb[:, h], ts[:, h],
                            mybir.ActivationFunctionType.Identity,
                            bias=bias, scale=1.0)
                        nc.sync.dma_start(
                            out=out[:, out_off + h * N:out_off + (h + 1) * N],
                            in_=o_sb[:, h])
                else:
                    # last tile: fused (t_v+bias)+t_e on DVE, half DMAs
                    # issued from the idle GpSimd queue as each half lands
                    nc.vector.scalar_tensor_tensor(
                        o_sb[:, 0], t[:, 0, 0], bv_sb, t[:, 1, 0],
                        op0=mybir.AluOpType.add, op1=mybir.AluOpType.add)
                    nc.gpsimd.dma_start(out=out[:, out_off:out_off + N],
                                        in_=o_sb[:, 0])
                    nc.vector.scalar_tensor_tensor(
                        o_sb[:, 1], t[:, 0, 1], be_sb, t[:, 1, 1],
                        op0=mybir.AluOpType.add, op1=mybir.AluOpType.add)
                    nc.scalar.dma_start(
                        out=out[:, out_off + N:out_off + 2 * N],
                        in_=o_sb[:, 1])
                out_off += 2 * N
    _split_multiwaits(nc)
    _hoist_first_dmas(nc, k=1 + len(NS))
    return nc


def _hoist_first_dmas(nc, k):
    """Move the first k wait-free SP DMA instructions to the very front of
    the first block, ahead of the framework preamble barriers, so their
    ~2.3us issue+transfer+semaphore latency hides under the ~7us engine
    spin-up window. Their semaphore updates are unchanged -- consumers
    still wait on the same counts."""
    blocks = nc.m.functions[0].blocks
    moved = []
    for b in blocks:
        insts = b.instructions
        keep = []
        for inst in insts:
            if (len(moved) < k and 'DMA' in str(inst.opcode)
                    and inst.engine == mybir.EngineType.SP
                    and (inst.sync_info is None
                         or not inst.sync_info.on_wait)):
                moved.append(inst)
            else:
                keep.append(inst)
        insts[:] = keep
        if len(moved) >= k:
            break
    blocks[0].instructions[:0] = moved
    return len(moved)


def _split_multiwaits(nc):
    """Split instructions carrying >1 sync wait into single-wait NoOps
    inserted just before them on the same engine queue."""
    n = 0
    for b in nc.m.functions[0].blocks:
        insts = b.instructions
        new = []
        for inst in insts:
            si = inst.sync_info
            if si is not None and si.on_wait and len(si.on_wait) > 1:
                waits = list(si.on_wait)
                for k, w in enumerate(waits[:-1]):
                    nop = mybir.InstNoOp(name=f"{inst.name}-sw{k}",
                                         ins=[], outs=[])
                    nop.engine = inst.engine
                    nop.sync_info = bass_rust.SyncInfo(on_wait=[w],
                                                       on_update=[])
                    nc.register_instruction(nop)
                    new.append(nop)
                    n += 1
                si.on_wait = [waits[-1]]
            new.append(inst)
        insts[:] = new
    return n


_NC = None


def _get_nc():
    global _NC
    if _NC is None:
        _NC = _build()
    return _NC


def _make_in_maps(v, e, w_vv, w_ve, w_ev, w_ee, bias_v, bias_e):
    cst = np.empty((D, CW), np.float16)
    cst[:, 0 * D:1 * D] = np.repeat(w_vv.reshape(D, 1), D, axis=1)
    cst[:, 1 * D:2 * D] = np.repeat(w_ev.reshape(D, 1), D, axis=1)
    cst[:, 2 * D:3 * D] = np.repeat(w_ve.reshape(D, 1), D, axis=1)
    cst[:, 3 * D:4 * D] = np.repeat(w_ee.reshape(D, 1), D, axis=1)
    cst[:, 4 * D] = bias_v.reshape(D)
    cst[:, 4 * D + 1] = bias_e.reshape(D)

    vT = np.ascontiguousarray(v.T).astype(np.float16)   # [D, B]
    eT = np.ascontiguousarray(e.T).astype(np.float16)
    in_maps = []
    for c in range(NCORES):
        xin = np.empty((D, CW + 2 * RPC), np.float16)
        xin[:, 0:CW] = cst
        base = c * RPC
        off = CW
        lo = base
        for N in NS:
            xin[:, off:off + N] = vT[:, lo:lo + N]
            xin[:, off + N:off + 2 * N] = eT[:, lo:lo + N]
            off += 2 * N
            lo += N
        in_maps.append({"xin": xin})
    return in_maps


def _run(in_maps, trace=False):
    return run_bass_kernel_spmd(_get_nc(), in_maps, list(range(NCORES)),
                                trace=trace)


def kernel(item_embedding, entity_embedding, w_vv, w_ve, w_ev, w_ee,
           bias_v, bias_e, _trace=False, _res_out=None):
    v = np.asarray(item_embedding, np.float32).reshape(B, D)
    e = np.asarray(entity_embedding, np.float32).reshape(B, D)
    in_maps = _make_in_maps(
        v, e,
        np.asarray(w_vv, np.float32), np.asarray(w_ve, np.float32),
        np.asarray(w_ev, np.float32), np.asarray(w_ee, np.float32),
        np.asarray(bias_v, np.float32), np.asarray(bias_e, np.float32))
    res = _run(in_maps, trace=_trace)
    if _res_out is not None:
        _res_out.append(res)
    item = np.empty((B, D, 1), np.float32)
    ent = np.empty((B, D, 1), np.float32)
    for c in range(NCORES):
        o = res.results[c]["out"]            # [D, 2*RPC] fp16
        base = c * RPC
        off = 0
        lo = base
        for N in NS:
            item[lo:lo + N, :, 0] = o[:, off:off + N].T
            ent[lo:lo + N, :, 0] = o[:, off + N:off + 2 * N].T
            off += 2 * N
            lo += N
    return (item, ent)
